# revision 16
# baseline (speedup 1.0000x reference)
"""Data-parallel GeneratedTreeClassifier forward on 8 NeuronCores (Bass/Tile).

Shards the batch dim of x (16384 -> 8 x 2048) across cores, replicates the
small tree params, runs a hand-written Bass/Tile kernel per core, and
gathers the full [16384, 512] output.

Math restructure (per tree t, decision i = 0..3, r = 1/(4 + d3 + eps)):
  out = leaf_norm @ (softmax(leaf_logits) * w)
      = r @ C + (r*d) @ G
  C_t = M_t0 + M_t2 + M_t4 + M_t6
  G_(t,i) = M_t(1+2i) - M_t(2+2i)  (i<3),   G_(t,3) = M_t7
which kills the per-tile leaf assembly + transpose and shrinks mm2's K
from 512 to 320.

Per-core device graph, processed in groups of 4 batch tiles (512 rows):
  xT   <- PE transpose (identity matmul) of bf16 x tiles  [128d, 4k, 512b]
  zT   = W @ x^T            (PE, j-major: 8 matmuls of N=512)
  d    = sigmoid(zT + bias) (ACT, bias per-partition)     [128j, 2, 512b]
  r    = 1/(4+d3+eps)       (DVE approx, partitions 64:128)
  e    = d * r              (DVE, bf16)   -> mm2 lhsT tiles T0, T1
  out  = [e; r] @ [G; C]    (PE, 3 matmuls of N=512 per batch tile)
  M    = softmax(leaf_logits)*w, C/G via pattern matmuls (one-time).
"""
import numpy as np
import ml_dtypes
from contextlib import ExitStack

import concourse.bass as bass
import concourse.tile as tile
from concourse import bacc, mybir

INPUT_DIM = 512
N_CLASSES = 512
N_TREES = 64
N_LEAVES = 8
N_INTERNAL = 7
PPT = N_INTERNAL * (INPUT_DIM + 1) + N_LEAVES * N_CLASSES
BATCH = 16384
N_CORES = 8
BSH = BATCH // N_CORES          # 2048 rows per core
NB = BSH // 128                 # 16 batch tiles per core
NG = NB // 4                    # 4 groups of 4 tiles
NW = N_INTERNAL * INPUT_DIM
EPS = 1e-8

F32 = mybir.dt.float32
BF16 = mybir.dt.bfloat16


def _emit(ctx: ExitStack, tc, xt, wT, bb, bb2, ll, wr, pc, pg, out):
    nc = tc.nc
    AF = mybir.ActivationFunctionType

    const = ctx.enter_context(tc.tile_pool(name="const", bufs=1))

    # Replicated params, resident in SBUF.
    wt_sb = const.tile([128, 4, 256], BF16)          # [d%128, dk, j]
    nc.sync.dma_start(wt_sb[:], wT.rearrange("(k p) j -> p k j", p=128))
    bias_sb = const.tile([128, 2], F32)              # [j%128, jb]
    nc.sync.dma_start(bias_sb[:], bb.rearrange("(jb p) one -> p (jb one)", p=128))
    bias2_sb = const.tile([128, 1], F32)             # b3 + ln(1.25) at 64:128
    nc.sync.dma_start(bias2_sb[64:128, :], bb2[:])
    pc_sb = const.tile([128, 4, 64], BF16)           # [tl%128, tlk, t]
    nc.sync.dma_start(pc_sb[:], pc.rearrange("(k p) t -> p k t", p=128))
    pg_sb = const.tile([128, 4, 256], BF16)          # [tl%128, tlk, j]
    nc.sync.dma_start(pg_sb[:], pg.rearrange("(k p) j -> p k j", p=128))
    m_sb = const.tile([128, 4, N_CLASSES], BF16)     # [tl%128, tlk, c]
    cg0 = const.tile([128, N_CLASSES], BF16)         # G rows (i0; i1)
    cg1 = const.tile([128, N_CLASSES], BF16)         # G rows (i2; i3)
    cg2 = const.tile([128, N_CLASSES], BF16)         # C rows at 64:128

    # M = softmax(leaf_logits, axis=-1) * w_tree   (rows tl = t*8 + l)
    ppool = ctx.enter_context(tc.tile_pool(name="prm", bufs=4))
    for k in range(4):
        llt = ppool.tile([128, N_CLASSES], F32, tag="llt")
        nc.sync.dma_start(llt[:], ll[k * 128:(k + 1) * 128, :])
        mx = ppool.tile([128, 1], F32, tag="mx")
        nc.vector.reduce_max(mx[:], llt[:], axis=mybir.AxisListType.X)
        nmx = ppool.tile([128, 1], F32, tag="nmx")
        nc.vector.tensor_scalar_mul(nmx[:], mx[:], -1.0)
        e = ppool.tile([128, N_CLASSES], F32, tag="e")
        s = ppool.tile([128, 1], F32, tag="s")
        nc.scalar.activation(e[:], llt[:], AF.Exp,
                             bias=nmx[:], scale=1.0, accum_out=s[:])
        rs = ppool.tile([128, 1], F32, tag="rs")
        nc.vector.reciprocal(rs[:], s[:])
        wrt = ppool.tile([128, 1], F32, tag="wrt")
        nc.sync.dma_start(wrt[:], wr[k * 128:(k + 1) * 128, :])
        sc = ppool.tile([128, 1], F32, tag="sc")
        nc.vector.tensor_tensor(sc[:], rs[:], wrt[:], op=mybir.AluOpType.mult)
        nc.vector.tensor_scalar_mul(m_sb[:, k, :], e[:], sc[:])

    spool = ctx.enter_context(tc.tile_pool(name="xT", bufs=1))
    dpool = ctx.enter_context(tc.tile_pool(name="work", bufs=3))
    epool = ctx.enter_context(tc.tile_pool(name="eT", bufs=3))
    opool = ctx.enter_context(tc.tile_pool(name="osb", bufs=6))
    zpp = ctx.enter_context(tc.tile_pool(name="zps", bufs=4, space="PSUM"))
    opp = ctx.enter_context(tc.tile_pool(name="ops", bufs=4, space="PSUM"))

    # Warm-up filler: dependency-free matmuls so the PE HAM clock-gate
    # reaches 8/8 (2.4 GHz) before the real pipeline starts.
    wpool = ctx.enter_context(tc.tile_pool(name="warm", bufs=1))
    wlhs = wpool.tile([128, 128], BF16)
    wrhs = wpool.tile([128, 512], BF16)
    nc.vector.memset(wlhs[:], 0.0)
    nc.vector.memset(wrhs[:], 0.0)
    wps = opp.tile([128, 512], F32, tag="o")
    for _ in range(10):
        nc.tensor.matmul(wps[:], lhsT=wlhs[:], rhs=wrhs[:],
                         start=True, stop=True)

    # One-time: C/G from M via host-provided 0/±1 pattern matrices.
    cg2ps = opp.tile([128, 512], F32, tag="o")
    for k in range(4):
        nc.tensor.matmul(cg2ps[64:128, :], lhsT=pc_sb[:, k, :],
                         rhs=m_sb[:, k, :], start=(k == 0), stop=(k == 3))
    nc.scalar.copy(cg2[64:128, :], cg2ps[64:128, :])
    cg0ps = opp.tile([128, 512], F32, tag="o")
    for k in range(4):
        nc.tensor.matmul(cg0ps[:], lhsT=pg_sb[:, k, 0:128],
                         rhs=m_sb[:, k, :], start=(k == 0), stop=(k == 3))
    nc.scalar.copy(cg0[:], cg0ps[:])
    cg1ps = opp.tile([128, 512], F32, tag="o")
    for k in range(4):
        nc.tensor.matmul(cg1ps[:], lhsT=pg_sb[:, k, 128:256],
                         rhs=m_sb[:, k, :], start=(k == 0), stop=(k == 3))
    nc.scalar.copy(cg1[:], cg1ps[:])

    # x^T resident in SBUF (pre-transposed on host): xT[p, k, b] = x[b, k*128+p]
    xT = spool.tile([128, 4, BSH], BF16)
    for k, eng in enumerate((nc.scalar, nc.gpsimd, nc.gpsimd, nc.scalar)):
        eng.dma_start(xT[:, k, :], xt[k * 128:(k + 1) * 128, :])

    for g in range(NG):
        # zT[j, b] = sum_d W[j, d] x[b, d]    j = i*64 + t, i-major
        gs = slice(g * 512, (g + 1) * 512)
        zt0 = zpp.tile([128, 512], F32, tag="zt")
        zt1 = zpp.tile([128, 512], F32, tag="zt")
        for jb, ztile in enumerate((zt0, zt1)):
            for k in range(4):
                nc.tensor.matmul(ztile[:],
                                 lhsT=wt_sb[:, k, jb * 128:(jb + 1) * 128],
                                 rhs=xT[:, k, gs],
                                 start=(k == 0), stop=(k == 3))
        d4 = dpool.tile([128, 2, 512], BF16, tag="d4")
        for jb, ztile in enumerate((zt0, zt1)):
            nc.scalar.activation(d4[:, jb, :], ztile[:], AF.Sigmoid,
                                 bias=bias_sb[:, jb:jb + 1])

        # r = 1/(4 + d3) = 1/4 - sigmoid(z3 + ln 1.25)/20   (exact identity)
        s3 = dpool.tile([128, 512], F32, tag="s3")
        nc.scalar.activation(s3[64:128, :], zt1[64:128, :], AF.Sigmoid,
                             bias=bias2_sb[64:128, :])
        rb = dpool.tile([128, 512], BF16, tag="rb")
        nc.vector.tensor_scalar(rb[64:128, :], s3[64:128, :], -0.05, 0.25,
                                op0=mybir.AluOpType.mult,
                                op1=mybir.AluOpType.add)
        # replicate r to all (i, jb) lanes:  r4[a*64+t, jb, b] = r[t, b]
        r4 = dpool.tile([128, 2, 512], BF16, tag="r4")
        for jb in range(2):
            for a in range(2):
                nc.gpsimd.dma_start(r4[a * 64:(a + 1) * 64, jb, :],
                                    rb[64:128, :])

        # e = d * r  -> lhsT tiles for mm2 (rows i*64+t match G rows)
        T0 = epool.tile([128, 512], BF16, tag="T0")
        T1 = epool.tile([128, 512], BF16, tag="T1")
        nc.vector.tensor_tensor(T0[:], d4[:, 0, :], r4[:, 0, :],
                                op=mybir.AluOpType.mult)
        nc.vector.tensor_tensor(T1[:], d4[:, 1, :], r4[:, 1, :],
                                op=mybir.AluOpType.mult)

        # out = e @ G + r @ C  per batch tile
        for bt in range(4):
            bs = slice(bt * 128, (bt + 1) * 128)
            ops = opp.tile([128, 512], F32, tag="o")
            nc.tensor.matmul(ops[:], lhsT=T0[:, bs], rhs=cg0[:],
                             start=True, stop=False)
            nc.tensor.matmul(ops[:], lhsT=T1[:, bs], rhs=cg1[:],
                             start=False, stop=False)
            nc.tensor.matmul(ops[:], lhsT=rb[64:128, bs], rhs=cg2[64:128, :],
                             start=False, stop=True)
            osb = opool.tile([128, 512], BF16, tag="osb")
            if bt % 2 == 0:
                nc.scalar.copy(osb[:], ops[:])
            else:
                nc.vector.tensor_copy(osb[:], ops[:])
            deng = nc.sync if bt % 2 == 0 else nc.gpsimd
            deng.dma_start(out[(4 * g + bt) * 128:(4 * g + bt + 1) * 128, :],
                           osb[:])


_NC = None
_RUNNER = None


def _get_nc():
    global _NC
    if _NC is None:
        nc = bacc.Bacc("TRN2", target_bir_lowering=False, debug=False)
        xt = nc.dram_tensor("xt", [INPUT_DIM, BSH], BF16, kind="ExternalInput")
        wT = nc.dram_tensor("wT", [INPUT_DIM, 256], BF16, kind="ExternalInput")
        bb = nc.dram_tensor("bb", [256, 1], F32, kind="ExternalInput")
        bb2 = nc.dram_tensor("bb2", [64, 1], F32, kind="ExternalInput")
        ll = nc.dram_tensor("ll", [512, N_CLASSES], F32, kind="ExternalInput")
        wr = nc.dram_tensor("wr", [512, 1], F32, kind="ExternalInput")
        pc = nc.dram_tensor("pc", [512, 64], BF16, kind="ExternalInput")
        pg = nc.dram_tensor("pg", [512, 256], BF16, kind="ExternalInput")
        out = nc.dram_tensor("out", [BSH, N_CLASSES], BF16, kind="ExternalOutput")
        with tile.TileContext(nc) as tc, ExitStack() as ctx:
            _emit(ctx, tc, xt.ap(), wT.ap(), bb.ap(), bb2.ap(), ll.ap(), wr.ap(),
                  pc.ap(), pg.ap(), out.ap())
        nc.compile()
        _NC = nc
    return _NC


def _get_runner():
    """Build the sharded PJRT executable ONCE (jit + NEFF compile are cached
    across kernel() calls; run_bass_kernel_spmd would re-trace every call)."""
    global _RUNNER
    if _RUNNER is None:
        import jax
        import jax.numpy as jnp
        from jax.sharding import Mesh, PartitionSpec, NamedSharding
        from jax.experimental.shard_map import shard_map
        from concourse import bass2jax

        nc = _get_nc()
        bass2jax.install_neuronx_cc_hook()

        part_name = (nc.partition_id_tensor.name
                     if nc.partition_id_tensor else None)
        in_names, out_names, out_avals = [], [], []
        for alloc in nc.m.functions[0].allocations:
            if not isinstance(alloc, mybir.MemoryLocationSet):
                continue
            name = alloc.memorylocations[0].name
            if alloc.kind == "ExternalInput":
                if name != part_name:
                    in_names.append(name)
            elif alloc.kind == "ExternalOutput":
                out_names.append(name)
                out_avals.append(jax.core.ShapedArray(
                    tuple(alloc.tensor_shape), mybir.dt.np(alloc.dtype)))
        n_params = len(in_names)
        all_names = tuple(in_names) + tuple(out_names)
        if part_name is not None:
            all_names = all_names + (part_name,)
        donate = tuple(range(n_params, n_params + len(out_names)))

        def _body(*args):
            operands = list(args)
            if part_name is not None:
                operands.append(bass2jax.partition_id_tensor())
            outs = bass2jax._bass_exec_p.bind(
                *operands,
                out_avals=tuple(out_avals),
                in_names=all_names,
                out_names=tuple(out_names),
                lowering_input_output_aliases=(),
                sim_require_finite=True,
                sim_require_nnan=True,
                nc=nc,
            )
            return tuple(outs)

        devices = jax.devices()[:N_CORES]
        mesh = Mesh(np.asarray(devices), ("core",))
        spec = PartitionSpec("core")
        fn = jax.jit(
            shard_map(_body, mesh=mesh,
                      in_specs=(spec,) * (n_params + len(out_names)),
                      out_specs=(spec,) * len(out_names), check_rep=False),
            donate_argnums=donate, keep_unused=True)
        zmk = jax.jit(
            lambda: jnp.zeros((N_CORES * BSH, N_CLASSES), ml_dtypes.bfloat16),
            out_shardings=NamedSharding(mesh, spec))
        _RUNNER = (fn, zmk, in_names)
    return _RUNNER


def _patterns():
    """0/±1 combination matrices: C = PC^T M, G = PG^T M (tl = 8t + l)."""
    pcm = np.zeros((512, 64), np.float32)
    pgm = np.zeros((512, 256), np.float32)
    for t in range(N_TREES):
        for l in (0, 2, 4, 6):
            pcm[8 * t + l, t] = 1.0
        for i in range(3):
            pgm[8 * t + 1 + 2 * i, i * 64 + t] = 1.0
            pgm[8 * t + 2 + 2 * i, i * 64 + t] = -1.0
        pgm[8 * t + 7, 3 * 64 + t] = 1.0
    return (pcm.astype(ml_dtypes.bfloat16), pgm.astype(ml_dtypes.bfloat16))


_PC, _PG = _patterns()


def _host_prep(x, tree_params, tree_weights):
    """Slice/layout the replicated params and cast x to bf16 (host-side)."""
    x = np.asarray(x, np.float32).astype(ml_dtypes.bfloat16)
    xt = np.empty((N_CORES * INPUT_DIM, BSH), ml_dtypes.bfloat16)
    for c in range(N_CORES):
        xt[c * INPUT_DIM:(c + 1) * INPUT_DIM] = x[c * BSH:(c + 1) * BSH].T
    p = np.asarray(tree_params, np.float32)[0].reshape(N_TREES, PPT)
    w = p[:, :NW].reshape(N_TREES, N_INTERNAL, INPUT_DIM)[:, :4, :]
    # j = i*64 + t (i-major)
    w_im = np.ascontiguousarray(w.transpose(1, 0, 2).reshape(256, INPUT_DIM))
    wT = np.ascontiguousarray(w_im.T).astype(ml_dtypes.bfloat16)
    bias = np.ascontiguousarray(
        p[:, NW:NW + N_INTERNAL][:, :4].T.reshape(256, 1))
    bias2 = np.ascontiguousarray(bias[192:256] + np.float32(np.log(1.25)))
    ll = np.ascontiguousarray(p[:, NW + N_INTERNAL:].reshape(512, N_CLASSES))
    wr = np.repeat(np.asarray(tree_weights, np.float32)[0], N_LEAVES)
    wr = np.ascontiguousarray(wr.reshape(512, 1))
    return xt, wT, bias, bias2, ll, wr


def kernel(x: np.ndarray, tree_params: np.ndarray,
           tree_weights: np.ndarray) -> np.ndarray:
    fn, zmk, in_names = _get_runner()
    xbf, wT, bias, bias2, ll, wr = _host_prep(x, tree_params, tree_weights)
    reps = {"xt": xbf,
            "wT": np.concatenate([wT] * N_CORES, 0),
            "bb": np.concatenate([bias] * N_CORES, 0),
            "bb2": np.concatenate([bias2] * N_CORES, 0),
            "ll": np.concatenate([ll] * N_CORES, 0),
            "wr": np.concatenate([wr] * N_CORES, 0),
            "pc": np.concatenate([_PC] * N_CORES, 0),
            "pg": np.concatenate([_PG] * N_CORES, 0)}
    args = [reps[n] for n in in_names] + [zmk()]
    outs = fn(*args)
    return np.asarray(outs[0]).astype(np.float32)
